# revision 42
# baseline (speedup 1.0000x reference)
"""Multi-head causal self-attention on 8 Trainium2 NeuronCores.

Sharding: tensor-parallel over heads (4 heads/core) x data-parallel over
batch (B=2): core c -> batch c//4, head-group c%4. Each core computes its
4 heads' attention plus a partial output projection; the host sums the 4
partials per batch element.

Layout strategy (per core):
  - x is fed pre-transposed (xT: [D, T]) so QKV projections produce
    qT/kT ([head_dim, T], head-dim on partitions) and v ([T, head_dim])
    directly, with no on-device transposes anywhere.
  - Startup: the critical-path DMA (wk/wq/xc-block0 in k-chunk order,
    rotated across three issue queues) goes out first so the QKV matmuls
    start as soon as chunk 0 lands and stream behind the DMA; wv / the
    other x blocks / wo / normalization constants follow in need-order.
  - Scores are computed transposed (k on partitions, q on free dim):
    psum[k, q] = kT_tile.T @ qT_block. Two heads run concurrently via
    row-tiled tile_position (dk=64 each) into one 2-bank psum tile, so
    one Exp activation covers the pair (halves ACT call overhead).
  - Softmax skips max-subtraction (scores are bounded well inside fp32
    exp range); exp runs on ScalarE with scale=1/sqrt(dk) folded in.
    Causal masking multiplies only diagonal tiles by a 0/1 mask, one
    head on VectorE and one on GpSimd.
  - P@V uses an M=65 stationary [v_head | ones] so the softmax
    denominators accumulate in psum row 64 of the same matmul.
  - Normalization: denominator rows are bounced via tiny DMAs into
    collector tiles (always read at partition base 0 — the custom-DVE
    fast reciprocal mishandles shifted bases on hardware), inverted
    with the fast Newton-Raphson reciprocal, broadcast across a full
    128-partition head-pair by a one-hot-selector matmul, and
    multiplied in with one DVE op per pair.
  - The PE clock gate (HAM) needs ~3.4us of sustained busy to reach
    2.4GHz; dummy matmuls on a zeroed scratch tile pre-warm it during
    the DMA-bound startup and bridge feed holes in the warm-up.
  - The exp chain paces the attention phase, so the PE stream is kept
    stall-free to hold the HAM clock at 2.4GHz: the m-loop is software
    pipelined (scores of step m+1 issue before P@V of step m), the two
    head pairs' blocks alternate, and QKV / output-projection matmul
    groups are interleaved into the attention phase as PE filler.
  - Matmul operands are bf16 (1 cycle/column on the PE; fp32r costs 2).
    Accumulation is fp32 in PSUM; denominators/reciprocals stay fp32.
  - y partials leave as bf16 (halves output DMA); the host sums in fp32.
"""

import sys

for _p in ("/opt/trn_rl_repo",):
    if _p not in sys.path:
        sys.path.append(_p)

import numpy as np

P = 128
T = 2048
D = 1024
OD = 256  # output dims per core = 4 heads x 64
DK = 64
NQ = 512  # q-block (psum free size)
N_CORES = 8

_CACHE = {}


def _build_nc(t=T, d=D, od=OD):
    import concourse.bass as bass
    import concourse.tile as tile
    from concourse import bacc, mybir

    f32 = mybir.dt.float32
    f32r = mybir.dt.float32r
    bf16 = mybir.dt.bfloat16

    kt = d // P        # k-tiles over d_model
    tt = t // P        # token tiles
    nb = t // NQ       # q blocks
    npair = od // P    # head pairs (2 heads per 128 partitions)
    dpb = NQ // P      # diagonal k-tiles per q block
    nh = od // DK      # heads per core

    nslotsA = 2 * npair * (nb - 1)  # (pair, j, head) slots with j < nb-1
    npairA = nslotsA // 2           # (pair, j) pair-slots with j < nb-1
    nslotsB = 2 * npair             # slots with j == nb-1
    nrows = max(nslotsA, 1)
    nrowsB = 32 * (nslotsB - 1) + 1  # batch-B rows live at partitions 32*i

    nc = bacc.Bacc("TRN2", target_bir_lowering=False, debug=False)

    xT = nc.dram_tensor("xT", [d, t], bf16, kind="ExternalInput")
    wqT = nc.dram_tensor("wqT", [d, od], bf16, kind="ExternalInput")
    wkT = nc.dram_tensor("wkT", [d, od], bf16, kind="ExternalInput")
    wvT = nc.dram_tensor("wvT", [d, od], bf16, kind="ExternalInput")
    woT = nc.dram_tensor("woT", [od, d], bf16, kind="ExternalInput")
    masks = nc.dram_tensor("masks", [P, 2 * P], bf16, kind="ExternalInput")
    emat = nc.dram_tensor("emat", [nrows, npairA * P], bf16, kind="ExternalInput")
    ematB = nc.dram_tensor("ematB", [33, P], bf16, kind="ExternalInput")
    y = nc.dram_tensor("y", [t, d], bf16, kind="ExternalOutput")

    Exp = mybir.ActivationFunctionType.Exp
    scale = 1.0 / float(np.sqrt(DK))

    with tile.TileContext(nc) as tc:
        with (
            tc.tile_pool(name="const", bufs=1) as cpool,
            tc.tile_pool(name="qk", bufs=2 * npair * nb) as qkpool,
            tc.tile_pool(name="vp", bufs=tt) as vpool,
            tc.tile_pool(name="ht", bufs=npair * nb) as hpool,
            tc.tile_pool(name="hu", bufs=npair * nb) as hupool,
            tc.tile_pool(name="work", bufs=8) as wpool,
            tc.tile_pool(name="psS", bufs=2, space="PSUM") as psS,
            tc.tile_pool(name="psH", bufs=2, space="PSUM") as psH,
            tc.tile_pool(name="psF", bufs=2, space="PSUM") as psF,
        ):
            # ---- HAM pre-warm: the PE clock gate needs ~3.4us of sustained
            # busy to unthrottle from 1.2 to 2.4 GHz, and the DMA-fed warmup
            # can't provide it. Stream dummy matmuls over a zeroed scratch
            # tile (no DMA dependency) so the clock is hot when real work
            # lands. ----
            warm_sb = cpool.tile([P, NQ], bf16, tag="warm")
            nc.vector.memset(warm_sb[:], 0.0)
            _wi = [0]

            def warm_mms(n):
                # dummy matmuls ride the psH ring (psF belongs to the QKV
                # stream); they bridge DMA-feed holes so the HAM clock gate
                # stays at 2.4 GHz through the warm-up
                ps = psH.tile([P, NQ], f32, tag="h", name=f"warm_{_wi[0]}")
                _wi[0] += 1
                for _ in range(n):
                    nc.tensor.matmul(ps[:], warm_sb[:, 0:P], warm_sb[:], start=True, stop=True)

            warm_mms(8)

            # ---- constant tiles (DMAs for the late-needed ones issue last) ----
            wo_sb = cpool.tile([P, npair * d], bf16, tag="wo")
            mask_sb = cpool.tile([P, 2 * P], bf16, tag="mask")
            emat_sb = cpool.tile([nrows, npairA * P], bf16, tag="emat")
            ematB_sb = cpool.tile([33, P], bf16, tag="ematB")

            # x and the QKV weights live in a scoped pool released after the
            # projections, freeing space for the attention phase.
            xpool = tc.alloc_tile_pool(name="xp", bufs=1)
            xc = [xpool.tile([P, kt * NQ], bf16, tag=f"xc{c}", name=f"xc_{c}") for c in range(nb)]
            wq_sb = xpool.tile([P, kt * od], bf16, tag="wq")
            wk_sb = xpool.tile([P, kt * od], bf16, tag="wk")
            wv_sb = xpool.tile([P, kt * od], bf16, tag="wv")

            _eng = [nc.sync, nc.gpsimd, nc.scalar]
            _ei = [0]

            def _issue(dst, src):
                _eng[_ei[0] % 3].dma_start(dst, src)
                _ei[0] += 1

            xTv = xT.rearrange("(k p) q -> p k q", p=P)
            wqv = wqT.rearrange("(k p) o -> p k o", p=P)
            wkv = wkT.rearrange("(k p) o -> p k o", p=P)
            wvv = wvT.rearrange("(k p) o -> p k o", p=P)
            xdst = [xc[c][:].rearrange("p (k q) -> p k q", q=NQ) for c in range(nb)]
            wqd = wq_sb[:].rearrange("p (k o) -> p k o", o=od)
            wkd = wk_sb[:].rearrange("p (k o) -> p k o", o=od)
            wvd = wv_sb[:].rearrange("p (k o) -> p k o", o=od)

            # Startup-critical stream rides the two HWDGE queues (sync, scalar
            # = SP/Activation hardware descriptor generators): finest chunks
            # first so the first matmuls fire as soon as chunk 0 lands, then
            # coarser batches to amortize the ~0.65us per-DMA issue cost.
            # gpsimd DMAs are SWDGE (descriptor generation occupies the Pool
            # engine ~1us per transfer), so it only carries tensors needed
            # after the warm-up: wv (needed ~5us in), the late x blocks, and
            # the output-side constants.
            # The Pool (gpsimd) SWDGE path stalls its queue for >10us once it
            # has a few transfers queued, so it only carries tensors needed
            # after ~40us (x2, x3, wo, norm constants). Everything the warmup
            # touches rides the two fast HWDGE queues.
            # Each queue admits only ~4 in-flight DMAs before issues stall on
            # transfer completions, so the startup-critical bytes must sit
            # inside that window: one finest-grain chunk to unblock the first
            # matmul, then one big transfer covering the rest of the tensor.
            # The warm-up is aggregate-bandwidth-bound (~2.5MB of critical
            # bytes), so the layout only needs consumption order: wk/x0
            # finest-first on the HWDGE queues, wq on the SWDGE queue, wv
            # before x1/x2, the rest late.
            assert nb == 4 and kt == 8
            nc.sync.dma_start(xdst[0][:, 0:1], xTv[:, 0:1, 0:NQ])
            nc.scalar.dma_start(wkd[:, 0:1], wkv[:, 0:1])
            nc.gpsimd.dma_start(wqd[:, 0:1], wqv[:, 0:1])
            nc.sync.dma_start(xdst[0][:, 1:2], xTv[:, 1:2, 0:NQ])
            nc.scalar.dma_start(wkd[:, 1:3], wkv[:, 1:3])
            nc.gpsimd.dma_start(wqd[:, 1:3], wqv[:, 1:3])
            nc.sync.dma_start(xdst[0][:, 2:4], xTv[:, 2:4, 0:NQ])
            nc.scalar.dma_start(wkd[:, 3:8], wkv[:, 3:8])
            nc.gpsimd.dma_start(wqd[:, 3:8], wqv[:, 3:8])
            nc.sync.dma_start(xdst[0][:, 4:6], xTv[:, 4:6, 0:NQ])
            nc.scalar.dma_start(wvd[:, 0:4], wvv[:, 0:4])
            nc.sync.dma_start(xdst[0][:, 6:8], xTv[:, 6:8, 0:NQ])
            nc.scalar.dma_start(wvd[:, 4:8], wvv[:, 4:8])
            nc.sync.dma_start(mask_sb[:], masks[:])
            nc.sync.dma_start(xdst[1][:, 0:4], xTv[:, 0:4, NQ:2 * NQ])
            nc.scalar.dma_start(xdst[1][:, 4:8], xTv[:, 4:8, NQ:2 * NQ])
            nc.sync.dma_start(xdst[2][:, 0:4], xTv[:, 0:4, 2 * NQ:3 * NQ])
            nc.scalar.dma_start(xdst[2][:, 4:8], xTv[:, 4:8, 2 * NQ:3 * NQ])
            nc.gpsimd.dma_start(xdst[3][:, 0:4], xTv[:, 0:4, 3 * NQ:4 * NQ])
            nc.gpsimd.dma_start(xdst[3][:, 4:8], xTv[:, 4:8, 3 * NQ:4 * NQ])
            for pp in range(npair):
                nc.gpsimd.dma_start(wo_sb[:, pp * d:(pp + 1) * d], woT[pp * P:(pp + 1) * P, :])
            nc.gpsimd.dma_start(emat_sb[:], emat[:])
            nc.gpsimd.dma_start(ematB_sb[:], ematB[:])

            # ---- persistent tiles ----
            qT = [[qkpool.tile([P, NQ], bf16, tag="qT", name=f"qT_{pp}_{n}") for n in range(nb)] for pp in range(npair)]
            kT = [[qkpool.tile([P, NQ], bf16, tag="kT", name=f"kT_{pp}_{n}") for n in range(nb)] for pp in range(npair)]
            v_sb = [vpool.tile([P, nh * (DK + 1)], bf16, tag="v", name=f"v_{tk}") for tk in range(tt)]
            hT = [[hpool.tile([P, NQ], bf16, tag="hT", name=f"hT_{pp}_{n}") for n in range(nb)] for pp in range(npair)]
            hu = {}

            sumsA = wpool.tile([max(nslotsA, 1), NQ], f32, tag="sumsA", bufs=1)
            # one [33, NQ] collector per pair so the custom-DVE
            # reciprocal_approx_fast always reads at partition offset 0
            # (its microcode mishandles shifted input bases on hardware)
            sumsB = [wpool.tile([33, NQ], f32, tag=f"sumsB{pp}", bufs=1, name=f"sumsB_{pp}") for pp in range(npair)]
            for pp in range(npair):
                nc.vector.memset(sumsB[pp][:], 1.0)
            batchA = []  # (pp, j, h) in collector-row order
            batchB = []

            # ---- emit helpers ----
            def emit_kq(pp, n, which=(0, 1)):
                for idx, (dst, w_sb) in enumerate(((kT, wk_sb), (qT, wq_sb))):
                    if idx not in which:
                        continue
                    ps = psF.tile([P, NQ], f32, tag="acc", name=f"kqps_{pp}_{n}_{idx}")
                    for k in range(kt):
                        nc.tensor.matmul(
                            ps[:],
                            w_sb[:, k * od + pp * P: k * od + (pp + 1) * P],
                            xc[n][:, k * NQ:(k + 1) * NQ],
                            start=(k == 0),
                            stop=(k == kt - 1),
                        )
                    nc.vector.tensor_copy(dst[pp][n][:], ps[:])

            def emit_v(tk):
                # each head's 64 v-columns are followed by a ones column so
                # the P@V matmul also accumulates the softmax denominator
                vv = v_sb[tk][:].rearrange("p (h c) -> p h c", c=DK + 1)
                nc.vector.memset(v_sb[tk][:], 1.0)
                ps = psF.tile([P, od], f32, tag="acc", name=f"vps_{tk}")
                for k in range(kt):
                    nc.tensor.matmul(
                        ps[:],
                        xc[tk // dpb][:, k * NQ + (tk % dpb) * P: k * NQ + (tk % dpb + 1) * P],
                        wv_sb[:, k * od:(k + 1) * od],
                        start=(k == 0),
                        stop=(k == kt - 1),
                    )
                nc.vector.tensor_copy(
                    vv[:, :, 0:DK],
                    ps[:].rearrange("p (h c) -> p h c", c=DK),
                )

            # ---- granule variants: one matmul (or one small op) per call so
            # the filler drip can fill sub-microsecond PE gaps between the
            # scores issue and the exp-gated P@V ----
            def kq_granules(pp, n, idx):
                dst, w_sb = ((kT, wk_sb), (qT, wq_sb))[idx]
                st = {}
                gs = []
                for k in range(kt):
                    def gk(k=k):
                        if k == 0:
                            st["ps"] = psF.tile([P, NQ], f32, tag="acc", name=f"kqps_{pp}_{n}_{idx}")
                        nc.tensor.matmul(
                            st["ps"][:],
                            w_sb[:, k * od + pp * P: k * od + (pp + 1) * P],
                            xc[n][:, k * NQ:(k + 1) * NQ],
                            start=(k == 0),
                            stop=(k == kt - 1),
                        )
                    gs.append(gk)
                gs.append(lambda: nc.vector.tensor_copy(dst[pp][n][:], st["ps"][:]))
                return gs

            def v_granules(tk):
                st = {}
                gs = []
                for k in range(kt):
                    def gk(k=k):
                        if k == 0:
                            nc.gpsimd.memset(v_sb[tk][:], 1.0)
                            st["ps"] = psF.tile([P, od], f32, tag="acc", name=f"vps_{tk}")
                        nc.tensor.matmul(
                            st["ps"][:],
                            xc[tk // dpb][:, k * NQ + (tk % dpb) * P: k * NQ + (tk % dpb + 1) * P],
                            wv_sb[:, k * od:(k + 1) * od],
                            start=(k == 0),
                            stop=(k == kt - 1),
                        )
                    gs.append(gk)

                def gc():
                    vv = v_sb[tk][:].rearrange("p (h c) -> p h c", c=DK + 1)
                    nc.vector.tensor_copy(vv[:, :, 0:DK], st["ps"][:].rearrange("p (h c) -> p h c", c=DK))
                gs.append(gc)
                return gs

            invA_holder = {}

            def emit_normA_recip(nslots):
                invf = wpool.tile([nslots, NQ], f32, tag="invAf", bufs=1, name="inv_Af")
                nc.vector.reciprocal_approx_fast(invf[:], sumsA[0:nslots, :])
                inv = wpool.tile([nslots, NQ], bf16, tag="invA", bufs=1, name="inv_A")
                nc.vector.tensor_copy(inv[:], invf[:])
                invA_holder["inv"] = inv

            def normA_pair_granules(ti, nslots):
                # pair-slot ti covers batchA rows 2*ti (head 0) and 2*ti+1
                # (head 1); one broadcast matmul + one mul handle all 128
                # partitions of the (pp, j) block
                pp, j, _ = batchA[2 * ti]
                st = {}

                def g0():
                    inv = invA_holder["inv"]
                    st["psb"] = psF.tile([P, NQ], f32, tag="acc", name=f"psb_A_{ti}")
                    nc.tensor.matmul(st["psb"][:], emat_sb[0:nslots, ti * P:(ti + 1) * P], inv[:], start=True, stop=True)

                def g1():
                    nc.vector.tensor_mul(hT[pp][j][:], hu[(pp, j)][:], st["psb"][:])
                return [g0, g1]

            def normB_granules(pp, ps_pool=None):
                # pair pp's two denominators live at rows 0 and 32 of its
                # own collector; ematB broadcasts row 0 -> partitions 0:64
                # and row 32 -> partitions 64:128
                st = {}

                def gr():
                    invf = wpool.tile([33, NQ], f32, tag="invBf", bufs=2, name=f"invBf_{pp}")
                    nc.vector.reciprocal_approx_fast(invf[:], sumsB[pp][0:33, :])
                    st["invB"] = wpool.tile([33, NQ], bf16, tag="invB", bufs=2, name=f"invB_{pp}")
                    nc.vector.tensor_copy(st["invB"][:], invf[:])

                def gm():
                    st["psb"] = (ps_pool or psF).tile([P, NQ], f32, tag="acc", name=f"psbB_{pp}")
                    nc.tensor.matmul(st["psb"][:], ematB_sb[0:33, :], st["invB"][:], start=True, stop=True)

                def gu():
                    nc.vector.tensor_mul(hT[pp][nb - 1][:], hu[(pp, nb - 1)][:], st["psb"][:])
                return [gr, gm, gu]

            obw = min(NQ, d)

            def oproj_granules(tk, use_act=False):
                gs = []
                for ob in range(d // obw):
                    st = {}

                    def gm(ob=ob, st=st):
                        st["psy"] = psF.tile([P, obw], f32, tag="acc", name=f"yps_{tk}_{ob}")
                        for pp in range(npair):
                            nc.tensor.matmul(
                                st["psy"][:],
                                hT[pp][tk // dpb][:, (tk % dpb) * P:(tk % dpb + 1) * P],
                                wo_sb[:, pp * d + ob * obw: pp * d + (ob + 1) * obw],
                                start=(pp == 0),
                                stop=(pp == npair - 1),
                            )

                    def gc(ob=ob, st=st):
                        ysb = wpool.tile([P, obw], bf16, tag="ysb", bufs=4)
                        if use_act:
                            # tail only: mid-phase the scalar queue is the
                            # exp pacer and must not carry copies
                            nc.scalar.copy(ysb[:], st["psy"][:])
                        else:
                            nc.vector.tensor_copy(ysb[:], st["psy"][:])
                        nc.sync.dma_start(y[tk * P:(tk + 1) * P, ob * obw:(ob + 1) * obw], ysb[:])
                    gs += [gm, gc]
                return gs

            # ---- warm-up: first-block QKV emitted eagerly; the scheduler
            # starts each piece as its DMA chunks land. Dummy matmuls woven
            # between the groups keep the PE busy across feed holes. ----
            emit_kq(0, 0)
            warm_mms(8)
            for tk in range(min(dpb, tt)):
                emit_v(tk)
                if tk < 3:
                    warm_mms(3)
            for pp in range(1, npair):
                emit_kq(pp, 0)
                warm_mms(2)

            # ---- attention m-step: both heads' scores land in one 2-bank
            # psum tile so a single Exp covers the pair; only the 128-wide
            # diagonal boundary needs the causal mask ----
            def emit_step(pp, j, m):
                dlt = m - dpb * j
                lo = max(dlt, 0) * P  # first live q column of this k-tile
                pss = psS.tile([P, 2 * NQ], f32, tag="acc", name=f"pss_{pp}_{j}_{m}")
                ps3 = pss[:].rearrange("p (h q) -> p h q", q=NQ)
                for h in range(2):
                    nc.tensor.matmul(
                        ps3[:, h, lo:],
                        kT[pp][m // dpb][h * DK:(h + 1) * DK, (m % dpb) * P:(m % dpb + 1) * P],
                        qT[pp][j][h * DK:(h + 1) * DK, lo:],
                        start=True,
                        stop=True,
                        tile_position=(h * DK, 0),
                    )
                e = wpool.tile([P, 2 * NQ], bf16, tag="exp", bufs=12)
                e3 = e[:].rearrange("p (h q) -> p h q", q=NQ)
                nc.scalar.activation(e3[:, :, lo:], ps3[:, :, lo:], Exp, bias=0.0, scale=scale)
                if dlt >= 0:
                    # one DVE op masks both heads (mask stored doubled)
                    nc.vector.tensor_mul(
                        e3[:, :, lo:lo + P],
                        e3[:, :, lo:lo + P],
                        mask_sb[:].rearrange("p (h q) -> p h q", q=P),
                    )
                return (m, lo, e3)

            def _emit_pv(pp, psh, nm, m, lo, e3):
                for h in range(2):
                    hh = 2 * pp + h
                    nc.tensor.matmul(
                        psh[h][0:DK + 1, lo:],
                        v_sb[m][:, hh * (DK + 1): (hh + 1) * (DK + 1)],
                        e3[:, h, lo:],
                        start=(m == 0),
                        stop=(m == nm - 1),
                    )

            def finish_block(pp, j, psh):
                # denominator rows first (they gate the batched reciprocal),
                # then the wider hu copies
                for h in range(2):
                    key = (pp, j, h)
                    if j < nb - 1:
                        row = len(batchA)
                        batchA.append(key)
                        stmp = wpool.tile([1, NQ], f32, tag="stmp", bufs=3)
                        nc.vector.tensor_copy(stmp[:], psh[h][DK:DK + 1, :])
                        # single-descriptor bounce: cheap on the SWDGE queue,
                        # and it keeps the sync queue free for y output tiles
                        nc.gpsimd.dma_start(sumsA[row:row + 1, :], stmp[:])
                    else:
                        batchB.append(key)
                        nc.vector.tensor_copy(sumsB[pp][32 * h:32 * h + 1, :], psh[h][DK:DK + 1, :])
                key = (pp, j)
                hu[key] = hupool.tile([P, NQ], bf16, tag="hu", name=f"hu_{pp}_{j}")
                for h in range(2):
                    if j == nb - 1 and pp == npair - 1 and h == 0:
                        # tail, unshifted half: ACT is idle there. (The h=1
                        # half needs a partition-shifted write, which only
                        # DVE handles on hardware.)
                        nc.scalar.copy(hu[key][0:DK, :], psh[h][0:DK, :])
                    else:
                        nc.vector.tensor_copy(hu[key][h * DK:(h + 1) * DK, :], psh[h][0:DK, :])

            released_x = False
            for j in range(nb):
                filler = []
                if j > 0:
                    # this block's own later v tiles (needed from m = dpb*j)
                    for tk in range(dpb * j, min(dpb * (j + 1), tt)):
                        filler.append(lambda tk=tk: emit_v(tk))
                if j < nb - 1:
                    for pp in range(npair):
                        filler.append(lambda pp=pp, n=j + 1: emit_kq(pp, n, (0,)))
                        filler.append(lambda pp=pp, n=j + 1: emit_kq(pp, n, (1,)))
                if j == nb - 1:
                    if nslotsA:
                        filler.append(lambda: emit_normA_recip(nslotsA))
                        # per q-block: its 2 normalization pair-slots, then
                        # the output-projection token blocks they unlock; the
                        # last two units stay reserved as PE work covering
                        # the tail's reciprocal window
                        for b in range(nb - 1):
                            for ti in (2 * b, 2 * b + 1):
                                filler += normA_pair_granules(ti, nslotsA)
                            for tk in range(dpb * b, dpb * (b + 1)):
                                if tk < dpb * (nb - 1) - 2:
                                    filler += oproj_granules(tk)
                nm = dpb * (j + 1)
                nsteps = npair * nm
                fstate = [0, 0, nsteps]  # steps done, fillers emitted, total
                for pp in range(npair):
                    psh = [psH.tile([P, NQ], f32, tag="h", name=f"psh_{pp}_{j}_{h}") for h in range(2)]
                    pending = None
                    for m in range(nm):
                        step = emit_step(pp, j, m)
                        if pending is not None:
                            _emit_pv(pp, psh, nm, *pending)
                        pending = step
                        fstate[0] += 1
                        while fstate[1] < len(filler) and fstate[1] < fstate[0] * len(filler) // max(fstate[2], 1):
                            filler[fstate[1]]()
                            fstate[1] += 1
                    _emit_pv(pp, psh, nm, *pending)
                    finish_block(pp, j, psh)
                    if j == nb - 1 and pp < npair - 1:
                        # queue this pair's normalization as filler so it
                        # drips into the next pair's steps
                        filler += normB_granules(pp)
                while fstate[1] < len(filler):
                    filler[fstate[1]]()
                    fstate[1] += 1
                if j >= nb - 2 and not released_x:
                    released_x = True
                    xpool.release()

            # ---- tail: last pair's normalization + last block's output
            # projection, software-pipelined 4 psum buffers deep. The first
            # pair-0 half-accumulations issue before the normalization chain
            # so the PE stays busy (and HAM-warm) while it resolves.
            units = [(tk, ob) for tk in range(dpb * (nb - 1), tt) for ob in range(d // obw)]
            psys = {}

            def tail_p0(u):
                tk, ob = units[u]
                pl, tg, w = ((psF, "acc", obw), (psH, "h", NQ))[u % 2]
                psy = pl.tile([P, w], f32, tag=tg, name=f"yt_{u}")
                nc.tensor.matmul(
                    psy[:, 0:obw],
                    hT[0][nb - 1][:, (tk % dpb) * P:(tk % dpb + 1) * P],
                    wo_sb[:, ob * obw:(ob + 1) * obw],
                    start=True, stop=False,
                )
                psys[u] = psy

            def tail_p1(u):
                tk, ob = units[u]
                psy = psys.pop(u)
                nc.tensor.matmul(
                    psy[:, 0:obw],
                    hT[1][nb - 1][:, (tk % dpb) * P:(tk % dpb + 1) * P],
                    wo_sb[:, d + ob * obw: d + (ob + 1) * obw],
                    start=False, stop=True,
                )
                ysb = wpool.tile([P, obw], bf16, tag="ysb", bufs=4)
                if u % 2:
                    nc.scalar.copy(ysb[:], psy[:, 0:obw])
                    # split the tail's output stream over both HWDGE queues
                    nc.scalar.dma_start(y[tk * P:(tk + 1) * P, ob * obw:(ob + 1) * obw], ysb[:])
                else:
                    nc.vector.tensor_copy(ysb[:], psy[:, 0:obw])
                    nc.sync.dma_start(y[tk * P:(tk + 1) * P, ob * obw:(ob + 1) * obw], ysb[:])

            # reserved block-(nb-2) oproj units first (they own the psF ring
            # before tail_p0 claims it), then the pair-0 halves — together
            # they keep the PE streaming through the reciprocal. ACT is idle
            # from here on, so their psum->sbuf copies ride the scalar queue.
            for tk in range(dpb * (nb - 1) - 2, dpb * (nb - 1)):
                for g in oproj_granules(tk, use_act=True):
                    g()
            for u in range(4):
                tail_p0(u)
            for g in normB_granules(npair - 1, ps_pool=psS):
                g()
            for u in range(4):
                tail_p1(u)
            # bridge the p1->p0 ring turnaround (a recurring ~0.9us PE gap
            # that re-throttles the clock gate right before the final
            # output-projection burst). Placed after the normalization chain
            # so it never delays the psb matmul that gates the p1 units.
            ps_wt = psS.tile([P, NQ], f32, tag="acc", name="warm_tail")
            for _ in range(4):
                nc.tensor.matmul(ps_wt[:], warm_sb[:, 0:P], warm_sb[:], start=True, stop=True)
            for u in range(4, len(units)):
                tail_p0(u)
            for u in range(4, len(units)):
                tail_p1(u)

    nc.compile()
    return nc


def _get_nc():
    if "nc" not in _CACHE:
        _CACHE["nc"] = _build_nc()
    return _CACHE["nc"]


def _emat_np(nrows):
    # pair-slot ti broadcasts row 2*ti across partitions 0:64 and row
    # 2*ti+1 across partitions 64:128 of its 128-wide column block
    import ml_dtypes
    e = np.zeros((nrows, (nrows // 2) * P), ml_dtypes.bfloat16)
    for ti in range(nrows // 2):
        e[2 * ti, ti * P: ti * P + DK] = 1.0
        e[2 * ti + 1, ti * P + DK: (ti + 1) * P] = 1.0
    return e


def _masks_np():
    import ml_dtypes
    kk = np.arange(P)[:, None]
    qq = np.arange(P)[None, :]
    m = (kk <= qq).astype(ml_dtypes.bfloat16)
    return np.concatenate([m, m], axis=1)


def _emat_rows(t=T, od=OD):
    nb = t // NQ
    npair = od // P
    return max(2 * npair * (nb - 1), 1)


def _ematB_np(t=T, od=OD):
    import ml_dtypes
    e = np.zeros((33, P), ml_dtypes.bfloat16)
    e[0, 0:DK] = 1.0
    e[32, DK:P] = 1.0
    return e


def make_in_maps(x, Wq, Wk, Wv, Wo):
    import ml_dtypes

    bf = ml_dtypes.bfloat16
    x = np.asarray(x, np.float32)
    msk = _masks_np()
    emat = _emat_np(_emat_rows())
    in_maps = []
    for c in range(N_CORES):
        b, g = c // (N_CORES // 2), c % (N_CORES // 2)
        hs = slice(OD * g, OD * (g + 1))
        in_maps.append({
            "xT": np.ascontiguousarray(x[b].T).astype(bf),
            "wqT": np.ascontiguousarray(np.asarray(Wq, np.float32)[hs, :].T).astype(bf),
            "wkT": np.ascontiguousarray(np.asarray(Wk, np.float32)[hs, :].T).astype(bf),
            "wvT": np.ascontiguousarray(np.asarray(Wv, np.float32)[hs, :].T).astype(bf),
            "woT": np.ascontiguousarray(np.asarray(Wo, np.float32)[:, hs].T).astype(bf),
            "masks": msk,
            "emat": emat,
            "ematB": _ematB_np(),
        })
    return in_maps


def combine_outputs(results):
    ng = N_CORES // 2
    out = np.empty((2, T, D), np.float32)
    for b in range(2):
        acc = results[b * ng]["y"].astype(np.float32)
        for g in range(1, ng):
            acc = acc + results[b * ng + g]["y"].astype(np.float32)
        out[b] = acc
    return out


def kernel(x, Wq, Wk, Wv, Wo):
    from concourse.bass_utils import run_bass_kernel_spmd

    nc = _get_nc()
    in_maps = make_in_maps(x, Wq, Wk, Wv, Wo)
    res = run_bass_kernel_spmd(nc, in_maps, list(range(N_CORES)))
    return combine_outputs(res.results)



# revision 44
# speedup vs baseline: 1.0251x; 1.0251x over previous
"""Multi-head causal self-attention on 8 Trainium2 NeuronCores.

Sharding: tensor-parallel over heads (4 heads/core) x data-parallel over
batch (B=2): core c -> batch c//4, head-group c%4. Each core computes its
4 heads' attention plus a partial output projection; the host sums the 4
partials per batch element.

Layout strategy (per core):
  - x is fed pre-transposed (xT: [D, T]) so QKV projections produce
    qT/kT ([head_dim, T], head-dim on partitions) and v ([T, head_dim])
    directly, with no on-device transposes anywhere.
  - Startup: the critical-path DMA (wk/wq/xc-block0 in k-chunk order,
    rotated across three issue queues) goes out first so the QKV matmuls
    start as soon as chunk 0 lands and stream behind the DMA; wv / the
    other x blocks / wo / normalization constants follow in need-order.
  - Scores are computed transposed (k on partitions, q on free dim):
    psum[k, q] = kT_tile.T @ qT_block. Two heads run concurrently via
    row-tiled tile_position (dk=64 each) into one 2-bank psum tile, so
    one Exp activation covers the pair (halves ACT call overhead).
  - Softmax skips max-subtraction (scores are bounded well inside fp32
    exp range); exp runs on ScalarE with scale=1/sqrt(dk) folded in.
    Causal masking multiplies only diagonal tiles by a 0/1 mask, one
    head on VectorE and one on GpSimd.
  - P@V uses an M=65 stationary [v_head | ones] so the softmax
    denominators accumulate in psum row 64 of the same matmul.
  - Normalization: denominator rows are bounced via tiny DMAs into
    collector tiles (always read at partition base 0 — the custom-DVE
    fast reciprocal mishandles shifted bases on hardware), inverted
    with the fast Newton-Raphson reciprocal, broadcast across a full
    128-partition head-pair by a one-hot-selector matmul, and
    multiplied in with one DVE op per pair.
  - The PE clock gate (HAM) needs ~3.4us of sustained busy to reach
    2.4GHz; dummy matmuls on a zeroed scratch tile pre-warm it during
    the DMA-bound startup and bridge feed holes in the warm-up.
  - The exp chain paces the attention phase, so the PE stream is kept
    stall-free to hold the HAM clock at 2.4GHz: the m-loop is software
    pipelined (scores of step m+1 issue before P@V of step m), the two
    head pairs' blocks alternate, and QKV / output-projection matmul
    groups are interleaved into the attention phase as PE filler.
  - Matmul operands are bf16 (1 cycle/column on the PE; fp32r costs 2).
    Accumulation is fp32 in PSUM; denominators/reciprocals stay fp32.
  - y partials leave as bf16 (halves output DMA); the host sums in fp32.
"""

import sys

for _p in ("/opt/trn_rl_repo",):
    if _p not in sys.path:
        sys.path.append(_p)

import numpy as np

P = 128
T = 2048
D = 1024
OD = 256  # output dims per core = 4 heads x 64
DK = 64
NQ = 512  # q-block (psum free size)
N_CORES = 8

_CACHE = {}


def _build_nc(t=T, d=D, od=OD):
    import concourse.bass as bass
    import concourse.tile as tile
    from concourse import bacc, mybir

    f32 = mybir.dt.float32
    f32r = mybir.dt.float32r
    bf16 = mybir.dt.bfloat16

    kt = d // P        # k-tiles over d_model
    tt = t // P        # token tiles
    nb = t // NQ       # q blocks
    npair = od // P    # head pairs (2 heads per 128 partitions)
    dpb = NQ // P      # diagonal k-tiles per q block
    nh = od // DK      # heads per core

    nslotsA = 2 * npair * (nb - 1)  # (pair, j, head) slots with j < nb-1
    npairA = nslotsA // 2           # (pair, j) pair-slots with j < nb-1
    nslotsB = 2 * npair             # slots with j == nb-1
    nrows = max(nslotsA, 1)
    nrowsB = 32 * (nslotsB - 1) + 1  # batch-B rows live at partitions 32*i

    nc = bacc.Bacc("TRN2", target_bir_lowering=False, debug=False)

    xT = nc.dram_tensor("xT", [d, t], bf16, kind="ExternalInput")
    wqT = nc.dram_tensor("wqT", [d, od], bf16, kind="ExternalInput")
    wkT = nc.dram_tensor("wkT", [d, od], bf16, kind="ExternalInput")
    wvT = nc.dram_tensor("wvT", [d, od], bf16, kind="ExternalInput")
    woT = nc.dram_tensor("woT", [od, d], bf16, kind="ExternalInput")
    masks = nc.dram_tensor("masks", [P, 2 * P], bf16, kind="ExternalInput")
    emat = nc.dram_tensor("emat", [nrows, npairA * P], bf16, kind="ExternalInput")
    ematB = nc.dram_tensor("ematB", [33, P], bf16, kind="ExternalInput")
    y = nc.dram_tensor("y", [t, d], bf16, kind="ExternalOutput")

    Exp = mybir.ActivationFunctionType.Exp
    scale = 1.0 / float(np.sqrt(DK))

    with tile.TileContext(nc) as tc:
        with (
            tc.tile_pool(name="const", bufs=1) as cpool,
            tc.tile_pool(name="qk", bufs=2 * npair * nb) as qkpool,
            tc.tile_pool(name="vp", bufs=tt) as vpool,
            tc.tile_pool(name="ht", bufs=npair * nb) as hpool,
            tc.tile_pool(name="hu", bufs=npair * nb) as hupool,
            tc.tile_pool(name="work", bufs=8) as wpool,
            tc.tile_pool(name="psS", bufs=2, space="PSUM") as psS,
            tc.tile_pool(name="psH", bufs=2, space="PSUM") as psH,
            tc.tile_pool(name="psF", bufs=2, space="PSUM") as psF,
        ):
            # ---- HAM pre-warm: the PE clock gate needs ~3.4us of sustained
            # busy to unthrottle from 1.2 to 2.4 GHz, and the DMA-fed warmup
            # can't provide it. Stream dummy matmuls over a zeroed scratch
            # tile (no DMA dependency) so the clock is hot when real work
            # lands. ----
            warm_sb = cpool.tile([P, NQ], bf16, tag="warm")
            nc.vector.memset(warm_sb[:], 0.0)
            _wi = [0]

            def warm_mms(n):
                # dummy matmuls ride the psH ring (psF belongs to the QKV
                # stream); they bridge DMA-feed holes so the HAM clock gate
                # stays at 2.4 GHz through the warm-up
                ps = psH.tile([P, NQ], f32, tag="h", name=f"warm_{_wi[0]}")
                _wi[0] += 1
                for _ in range(n):
                    nc.tensor.matmul(ps[:], warm_sb[:, 0:P], warm_sb[:], start=True, stop=True)

            warm_mms(8)

            # ---- constant tiles (DMAs for the late-needed ones issue last) ----
            wo_sb = cpool.tile([P, npair * d], bf16, tag="wo")
            mask_sb = cpool.tile([P, 2 * P], bf16, tag="mask")
            emat_sb = cpool.tile([nrows, npairA * P], bf16, tag="emat")
            ematB_sb = cpool.tile([33, P], bf16, tag="ematB")

            # x and the QKV weights live in a scoped pool released after the
            # projections, freeing space for the attention phase.
            xpool = tc.alloc_tile_pool(name="xp", bufs=1)
            xc = [xpool.tile([P, kt * NQ], bf16, tag=f"xc{c}", name=f"xc_{c}") for c in range(nb)]
            wq_sb = xpool.tile([P, kt * od], bf16, tag="wq")
            wk_sb = xpool.tile([P, kt * od], bf16, tag="wk")
            wv_sb = xpool.tile([P, kt * od], bf16, tag="wv")

            _eng = [nc.sync, nc.gpsimd, nc.scalar]
            _ei = [0]

            def _issue(dst, src):
                _eng[_ei[0] % 3].dma_start(dst, src)
                _ei[0] += 1

            xTv = xT.rearrange("(k p) q -> p k q", p=P)
            wqv = wqT.rearrange("(k p) o -> p k o", p=P)
            wkv = wkT.rearrange("(k p) o -> p k o", p=P)
            wvv = wvT.rearrange("(k p) o -> p k o", p=P)
            xdst = [xc[c][:].rearrange("p (k q) -> p k q", q=NQ) for c in range(nb)]
            wqd = wq_sb[:].rearrange("p (k o) -> p k o", o=od)
            wkd = wk_sb[:].rearrange("p (k o) -> p k o", o=od)
            wvd = wv_sb[:].rearrange("p (k o) -> p k o", o=od)

            # Startup-critical stream rides the two HWDGE queues (sync, scalar
            # = SP/Activation hardware descriptor generators): finest chunks
            # first so the first matmuls fire as soon as chunk 0 lands, then
            # coarser batches to amortize the ~0.65us per-DMA issue cost.
            # gpsimd DMAs are SWDGE (descriptor generation occupies the Pool
            # engine ~1us per transfer), so it only carries tensors needed
            # after the warm-up: wv (needed ~5us in), the late x blocks, and
            # the output-side constants.
            # The Pool (gpsimd) SWDGE path stalls its queue for >10us once it
            # has a few transfers queued, so it only carries tensors needed
            # after ~40us (x2, x3, wo, norm constants). Everything the warmup
            # touches rides the two fast HWDGE queues.
            # Each queue admits only ~4 in-flight DMAs before issues stall on
            # transfer completions, so the startup-critical bytes must sit
            # inside that window: one finest-grain chunk to unblock the first
            # matmul, then one big transfer covering the rest of the tensor.
            # The warm-up is aggregate-bandwidth-bound (~2.5MB of critical
            # bytes), so the layout only needs consumption order: wk/x0
            # finest-first on the HWDGE queues, wq on the SWDGE queue, wv
            # before x1/x2, the rest late.
            assert nb == 4 and kt == 8
            nc.sync.dma_start(xdst[0][:, 0:1], xTv[:, 0:1, 0:NQ])
            nc.scalar.dma_start(wkd[:, 0:1], wkv[:, 0:1])
            nc.gpsimd.dma_start(wqd[:, 0:1], wqv[:, 0:1])
            nc.sync.dma_start(xdst[0][:, 1:2], xTv[:, 1:2, 0:NQ])
            nc.scalar.dma_start(wkd[:, 1:3], wkv[:, 1:3])
            nc.gpsimd.dma_start(wqd[:, 1:3], wqv[:, 1:3])
            nc.sync.dma_start(xdst[0][:, 2:4], xTv[:, 2:4, 0:NQ])
            nc.scalar.dma_start(wkd[:, 3:8], wkv[:, 3:8])
            nc.gpsimd.dma_start(wqd[:, 3:8], wqv[:, 3:8])
            nc.sync.dma_start(xdst[0][:, 4:6], xTv[:, 4:6, 0:NQ])
            nc.scalar.dma_start(wvd[:, 0:4], wvv[:, 0:4])
            nc.sync.dma_start(xdst[0][:, 6:8], xTv[:, 6:8, 0:NQ])
            nc.scalar.dma_start(wvd[:, 4:8], wvv[:, 4:8])
            nc.sync.dma_start(mask_sb[:], masks[:])
            nc.sync.dma_start(xdst[1][:, 0:4], xTv[:, 0:4, NQ:2 * NQ])
            nc.scalar.dma_start(xdst[1][:, 4:8], xTv[:, 4:8, NQ:2 * NQ])
            nc.sync.dma_start(xdst[2][:, 0:4], xTv[:, 0:4, 2 * NQ:3 * NQ])
            nc.scalar.dma_start(xdst[2][:, 4:8], xTv[:, 4:8, 2 * NQ:3 * NQ])
            nc.gpsimd.dma_start(xdst[3][:, 0:4], xTv[:, 0:4, 3 * NQ:4 * NQ])
            nc.gpsimd.dma_start(xdst[3][:, 4:8], xTv[:, 4:8, 3 * NQ:4 * NQ])
            for pp in range(npair):
                nc.gpsimd.dma_start(wo_sb[:, pp * d:(pp + 1) * d], woT[pp * P:(pp + 1) * P, :])
            nc.gpsimd.dma_start(emat_sb[:], emat[:])
            nc.gpsimd.dma_start(ematB_sb[:], ematB[:])

            # ---- persistent tiles ----
            qT = [[qkpool.tile([P, NQ], bf16, tag="qT", name=f"qT_{pp}_{n}") for n in range(nb)] for pp in range(npair)]
            kT = [[qkpool.tile([P, NQ], bf16, tag="kT", name=f"kT_{pp}_{n}") for n in range(nb)] for pp in range(npair)]
            v_sb = [vpool.tile([P, nh * (DK + 1)], bf16, tag="v", name=f"v_{tk}") for tk in range(tt)]
            hT = [[hpool.tile([P, NQ], bf16, tag="hT", name=f"hT_{pp}_{n}") for n in range(nb)] for pp in range(npair)]
            hu = {}

            sumsA = wpool.tile([max(nslotsA, 1), NQ], f32, tag="sumsA", bufs=1)
            # one [33, NQ] collector per pair so the custom-DVE
            # reciprocal_approx_fast always reads at partition offset 0
            # (its microcode mishandles shifted input bases on hardware)
            sumsB = [wpool.tile([33, NQ], f32, tag=f"sumsB{pp}", bufs=1, name=f"sumsB_{pp}") for pp in range(npair)]
            for pp in range(npair):
                nc.vector.memset(sumsB[pp][:], 1.0)
            batchA = []  # (pp, j, h) in collector-row order
            batchB = []

            # ---- emit helpers ----
            def emit_kq(pp, n, which=(0, 1)):
                for idx, (dst, w_sb) in enumerate(((kT, wk_sb), (qT, wq_sb))):
                    if idx not in which:
                        continue
                    ps = psF.tile([P, NQ], f32, tag="acc", name=f"kqps_{pp}_{n}_{idx}")
                    for k in range(kt):
                        nc.tensor.matmul(
                            ps[:],
                            w_sb[:, k * od + pp * P: k * od + (pp + 1) * P],
                            xc[n][:, k * NQ:(k + 1) * NQ],
                            start=(k == 0),
                            stop=(k == kt - 1),
                        )
                    nc.vector.tensor_copy(dst[pp][n][:], ps[:])

            def emit_v(tk):
                # each head's 64 v-columns are followed by a ones column so
                # the P@V matmul also accumulates the softmax denominator
                vv = v_sb[tk][:].rearrange("p (h c) -> p h c", c=DK + 1)
                nc.vector.memset(v_sb[tk][:], 1.0)
                ps = psF.tile([P, od], f32, tag="acc", name=f"vps_{tk}")
                for k in range(kt):
                    nc.tensor.matmul(
                        ps[:],
                        xc[tk // dpb][:, k * NQ + (tk % dpb) * P: k * NQ + (tk % dpb + 1) * P],
                        wv_sb[:, k * od:(k + 1) * od],
                        start=(k == 0),
                        stop=(k == kt - 1),
                    )
                nc.vector.tensor_copy(
                    vv[:, :, 0:DK],
                    ps[:].rearrange("p (h c) -> p h c", c=DK),
                )

            # ---- granule variants: one matmul (or one small op) per call so
            # the filler drip can fill sub-microsecond PE gaps between the
            # scores issue and the exp-gated P@V ----
            def kq_granules(pp, n, idx):
                dst, w_sb = ((kT, wk_sb), (qT, wq_sb))[idx]
                st = {}
                gs = []
                for k in range(kt):
                    def gk(k=k):
                        if k == 0:
                            st["ps"] = psF.tile([P, NQ], f32, tag="acc", name=f"kqps_{pp}_{n}_{idx}")
                        nc.tensor.matmul(
                            st["ps"][:],
                            w_sb[:, k * od + pp * P: k * od + (pp + 1) * P],
                            xc[n][:, k * NQ:(k + 1) * NQ],
                            start=(k == 0),
                            stop=(k == kt - 1),
                        )
                    gs.append(gk)
                gs.append(lambda: nc.vector.tensor_copy(dst[pp][n][:], st["ps"][:]))
                return gs

            def v_granules(tk):
                st = {}
                gs = []
                for k in range(kt):
                    def gk(k=k):
                        if k == 0:
                            nc.gpsimd.memset(v_sb[tk][:], 1.0)
                            st["ps"] = psF.tile([P, od], f32, tag="acc", name=f"vps_{tk}")
                        nc.tensor.matmul(
                            st["ps"][:],
                            xc[tk // dpb][:, k * NQ + (tk % dpb) * P: k * NQ + (tk % dpb + 1) * P],
                            wv_sb[:, k * od:(k + 1) * od],
                            start=(k == 0),
                            stop=(k == kt - 1),
                        )
                    gs.append(gk)

                def gc():
                    vv = v_sb[tk][:].rearrange("p (h c) -> p h c", c=DK + 1)
                    nc.vector.tensor_copy(vv[:, :, 0:DK], st["ps"][:].rearrange("p (h c) -> p h c", c=DK))
                gs.append(gc)
                return gs

            invA_holder = {}

            def emit_normA_recip(nslots):
                invf = wpool.tile([nslots, NQ], f32, tag="invAf", bufs=1, name="inv_Af")
                nc.vector.reciprocal_approx_fast(invf[:], sumsA[0:nslots, :])
                inv = wpool.tile([nslots, NQ], bf16, tag="invA", bufs=1, name="inv_A")
                nc.vector.tensor_copy(inv[:], invf[:])
                invA_holder["inv"] = inv

            def normA_pair_granules(ti, nslots):
                # pair-slot ti covers batchA rows 2*ti (head 0) and 2*ti+1
                # (head 1); one broadcast matmul + one mul handle all 128
                # partitions of the (pp, j) block
                pp, j, _ = batchA[2 * ti]
                st = {}

                def g0():
                    inv = invA_holder["inv"]
                    st["psb"] = psF.tile([P, NQ], f32, tag="acc", name=f"psb_A_{ti}")
                    nc.tensor.matmul(st["psb"][:], emat_sb[0:nslots, ti * P:(ti + 1) * P], inv[:], start=True, stop=True)

                def g1():
                    nc.vector.tensor_mul(hT[pp][j][:], hu[(pp, j)][:], st["psb"][:])
                return [g0, g1]

            def normB_granules(pp, ps_pool=None):
                # pair pp's two denominators live at rows 0 and 32 of its
                # own collector; ematB broadcasts row 0 -> partitions 0:64
                # and row 32 -> partitions 64:128
                st = {}

                def gr():
                    invf = wpool.tile([33, NQ], f32, tag="invBf", bufs=2, name=f"invBf_{pp}")
                    nc.vector.reciprocal_approx_fast(invf[:], sumsB[pp][0:33, :])
                    st["invB"] = wpool.tile([33, NQ], bf16, tag="invB", bufs=2, name=f"invB_{pp}")
                    nc.vector.tensor_copy(st["invB"][:], invf[:])

                def gm():
                    st["psb"] = (ps_pool or psF).tile([P, NQ], f32, tag="acc", name=f"psbB_{pp}")
                    nc.tensor.matmul(st["psb"][:], ematB_sb[0:33, :], st["invB"][:], start=True, stop=True)

                def gu():
                    nc.vector.tensor_mul(hT[pp][nb - 1][:], hu[(pp, nb - 1)][:], st["psb"][:])
                return [gr, gm, gu]

            obw = min(NQ, d)

            def oproj_granules(tk, use_act=False):
                gs = []
                for ob in range(d // obw):
                    st = {}

                    def gm(ob=ob, st=st):
                        st["psy"] = psF.tile([P, obw], f32, tag="acc", name=f"yps_{tk}_{ob}")
                        for pp in range(npair):
                            nc.tensor.matmul(
                                st["psy"][:],
                                hT[pp][tk // dpb][:, (tk % dpb) * P:(tk % dpb + 1) * P],
                                wo_sb[:, pp * d + ob * obw: pp * d + (ob + 1) * obw],
                                start=(pp == 0),
                                stop=(pp == npair - 1),
                            )

                    def gc(ob=ob, st=st):
                        ysb = wpool.tile([P, obw], bf16, tag="ysb", bufs=4)
                        if use_act:
                            # tail only: mid-phase the scalar queue is the
                            # exp pacer and must not carry copies
                            nc.scalar.copy(ysb[:], st["psy"][:])
                        else:
                            nc.vector.tensor_copy(ysb[:], st["psy"][:])
                        nc.sync.dma_start(y[tk * P:(tk + 1) * P, ob * obw:(ob + 1) * obw], ysb[:])
                    gs += [gm, gc]
                return gs

            # ---- warm-up: first-block QKV emitted eagerly; the scheduler
            # starts each piece as its DMA chunks land. Dummy matmuls woven
            # between the groups keep the PE busy across feed holes. ----
            emit_kq(0, 0)
            warm_mms(4)
            for tk in range(min(dpb, tt)):
                emit_v(tk)
                if tk < 3:
                    warm_mms(2)
            for pp in range(1, npair):
                emit_kq(pp, 0)

            # ---- attention m-step: both heads' scores land in one 2-bank
            # psum tile so a single Exp covers the pair; only the 128-wide
            # diagonal boundary needs the causal mask ----
            def emit_step(pp, j, m):
                dlt = m - dpb * j
                lo = max(dlt, 0) * P  # first live q column of this k-tile
                pss = psS.tile([P, 2 * NQ], f32, tag="acc", name=f"pss_{pp}_{j}_{m}")
                ps3 = pss[:].rearrange("p (h q) -> p h q", q=NQ)
                for h in range(2):
                    nc.tensor.matmul(
                        ps3[:, h, lo:],
                        kT[pp][m // dpb][h * DK:(h + 1) * DK, (m % dpb) * P:(m % dpb + 1) * P],
                        qT[pp][j][h * DK:(h + 1) * DK, lo:],
                        start=True,
                        stop=True,
                        tile_position=(h * DK, 0),
                    )
                e = wpool.tile([P, 2 * NQ], bf16, tag="exp", bufs=12)
                e3 = e[:].rearrange("p (h q) -> p h q", q=NQ)
                nc.scalar.activation(e3[:, :, lo:], ps3[:, :, lo:], Exp, bias=0.0, scale=scale)
                if dlt >= 0:
                    # one DVE op masks both heads (mask stored doubled)
                    nc.vector.tensor_mul(
                        e3[:, :, lo:lo + P],
                        e3[:, :, lo:lo + P],
                        mask_sb[:].rearrange("p (h q) -> p h q", q=P),
                    )
                return (m, lo, e3)

            def _emit_pv(pp, psh, nm, m, lo, e3):
                for h in range(2):
                    hh = 2 * pp + h
                    nc.tensor.matmul(
                        psh[h][0:DK + 1, lo:],
                        v_sb[m][:, hh * (DK + 1): (hh + 1) * (DK + 1)],
                        e3[:, h, lo:],
                        start=(m == 0),
                        stop=(m == nm - 1),
                    )

            def finish_block(pp, j, psh):
                # denominator rows first (they gate the batched reciprocal),
                # then the wider hu copies
                for h in range(2):
                    key = (pp, j, h)
                    if j < nb - 1:
                        row = len(batchA)
                        batchA.append(key)
                        stmp = wpool.tile([1, NQ], f32, tag="stmp", bufs=3)
                        nc.vector.tensor_copy(stmp[:], psh[h][DK:DK + 1, :])
                        # single-descriptor bounce: cheap on the SWDGE queue,
                        # and it keeps the sync queue free for y output tiles
                        nc.gpsimd.dma_start(sumsA[row:row + 1, :], stmp[:])
                    else:
                        batchB.append(key)
                        nc.vector.tensor_copy(sumsB[pp][32 * h:32 * h + 1, :], psh[h][DK:DK + 1, :])
                key = (pp, j)
                hu[key] = hupool.tile([P, NQ], bf16, tag="hu", name=f"hu_{pp}_{j}")
                for h in range(2):
                    if j == nb - 1 and pp == npair - 1 and h == 0:
                        # tail, unshifted half: ACT is idle there. (The h=1
                        # half needs a partition-shifted write, which only
                        # DVE handles on hardware.)
                        nc.scalar.copy(hu[key][0:DK, :], psh[h][0:DK, :])
                    else:
                        nc.vector.tensor_copy(hu[key][h * DK:(h + 1) * DK, :], psh[h][0:DK, :])

            released_x = False
            for j in range(nb):
                filler = []
                if j > 0:
                    # this block's own later v tiles (needed from m = dpb*j)
                    for tk in range(dpb * j, min(dpb * (j + 1), tt)):
                        filler.append(lambda tk=tk: emit_v(tk))
                if j < nb - 1:
                    for pp in range(npair):
                        filler.append(lambda pp=pp, n=j + 1: emit_kq(pp, n, (0,)))
                        filler.append(lambda pp=pp, n=j + 1: emit_kq(pp, n, (1,)))
                if j == nb - 1:
                    if nslotsA:
                        filler.append(lambda: emit_normA_recip(nslotsA))
                        # per q-block: its 2 normalization pair-slots, then
                        # the output-projection token blocks they unlock; the
                        # last two units stay reserved as PE work covering
                        # the tail's reciprocal window
                        for b in range(nb - 1):
                            for ti in (2 * b, 2 * b + 1):
                                filler += normA_pair_granules(ti, nslotsA)
                            for tk in range(dpb * b, dpb * (b + 1)):
                                if tk < dpb * (nb - 1) - 2:
                                    filler += oproj_granules(tk)
                nm = dpb * (j + 1)
                nsteps = npair * nm
                fstate = [0, 0, nsteps]  # steps done, fillers emitted, total
                for pp in range(npair):
                    psh = [psH.tile([P, NQ], f32, tag="h", name=f"psh_{pp}_{j}_{h}") for h in range(2)]
                    pending = None
                    for m in range(nm):
                        step = emit_step(pp, j, m)
                        # drip fillers between the scores issue and the
                        # exp-gated P@V so the PE crosses the exp latency
                        # without an exposed pipeline-drain boundary
                        fstate[0] += 1
                        while fstate[1] < len(filler) and fstate[1] < fstate[0] * len(filler) // max(fstate[2], 1):
                            filler[fstate[1]]()
                            fstate[1] += 1
                        if pending is not None:
                            _emit_pv(pp, psh, nm, *pending)
                        pending = step
                    _emit_pv(pp, psh, nm, *pending)
                    finish_block(pp, j, psh)
                    if j == nb - 1 and pp < npair - 1:
                        # queue this pair's normalization as filler so it
                        # drips into the next pair's steps
                        filler += normB_granules(pp)
                while fstate[1] < len(filler):
                    filler[fstate[1]]()
                    fstate[1] += 1
                if j >= nb - 2 and not released_x:
                    released_x = True
                    xpool.release()

            # ---- tail: last pair's normalization + last block's output
            # projection, software-pipelined 4 psum buffers deep. The first
            # pair-0 half-accumulations issue before the normalization chain
            # so the PE stays busy (and HAM-warm) while it resolves.
            units = [(tk, ob) for tk in range(dpb * (nb - 1), tt) for ob in range(d // obw)]
            psys = {}

            def tail_p0(u):
                tk, ob = units[u]
                pl, tg, w = ((psF, "acc", obw), (psH, "h", NQ))[u % 2]
                psy = pl.tile([P, w], f32, tag=tg, name=f"yt_{u}")
                nc.tensor.matmul(
                    psy[:, 0:obw],
                    hT[0][nb - 1][:, (tk % dpb) * P:(tk % dpb + 1) * P],
                    wo_sb[:, ob * obw:(ob + 1) * obw],
                    start=True, stop=False,
                )
                psys[u] = psy

            def tail_p1(u):
                tk, ob = units[u]
                psy = psys.pop(u)
                nc.tensor.matmul(
                    psy[:, 0:obw],
                    hT[1][nb - 1][:, (tk % dpb) * P:(tk % dpb + 1) * P],
                    wo_sb[:, d + ob * obw: d + (ob + 1) * obw],
                    start=False, stop=True,
                )
                ysb = wpool.tile([P, obw], bf16, tag="ysb", bufs=4)
                if u % 2:
                    nc.scalar.copy(ysb[:], psy[:, 0:obw])
                    # split the tail's output stream over both HWDGE queues
                    nc.scalar.dma_start(y[tk * P:(tk + 1) * P, ob * obw:(ob + 1) * obw], ysb[:])
                else:
                    nc.vector.tensor_copy(ysb[:], psy[:, 0:obw])
                    nc.sync.dma_start(y[tk * P:(tk + 1) * P, ob * obw:(ob + 1) * obw], ysb[:])

            # reserved block-(nb-2) oproj units first (they own the psF ring
            # before tail_p0 claims it), then the pair-0 halves — together
            # they keep the PE streaming through the reciprocal. ACT is idle
            # from here on, so their psum->sbuf copies ride the scalar queue.
            for tk in range(dpb * (nb - 1) - 2, dpb * (nb - 1)):
                for g in oproj_granules(tk, use_act=True):
                    g()
            for u in range(4):
                tail_p0(u)
            for g in normB_granules(npair - 1, ps_pool=psS):
                g()
            for u in range(4):
                tail_p1(u)
            # bridge the p1->p0 ring turnaround (a recurring ~0.9us PE gap
            # that re-throttles the clock gate right before the final
            # output-projection burst). Placed after the normalization chain
            # so it never delays the psb matmul that gates the p1 units.
            ps_wt = psS.tile([P, NQ], f32, tag="acc", name="warm_tail")
            for _ in range(4):
                nc.tensor.matmul(ps_wt[:], warm_sb[:, 0:P], warm_sb[:], start=True, stop=True)
            for u in range(4, len(units)):
                tail_p0(u)
            for u in range(4, len(units)):
                tail_p1(u)

    nc.compile()
    return nc


def _get_nc():
    if "nc" not in _CACHE:
        _CACHE["nc"] = _build_nc()
    return _CACHE["nc"]


def _emat_np(nrows):
    # pair-slot ti broadcasts row 2*ti across partitions 0:64 and row
    # 2*ti+1 across partitions 64:128 of its 128-wide column block
    import ml_dtypes
    e = np.zeros((nrows, (nrows // 2) * P), ml_dtypes.bfloat16)
    for ti in range(nrows // 2):
        e[2 * ti, ti * P: ti * P + DK] = 1.0
        e[2 * ti + 1, ti * P + DK: (ti + 1) * P] = 1.0
    return e


def _masks_np():
    import ml_dtypes
    kk = np.arange(P)[:, None]
    qq = np.arange(P)[None, :]
    m = (kk <= qq).astype(ml_dtypes.bfloat16)
    return np.concatenate([m, m], axis=1)


def _emat_rows(t=T, od=OD):
    nb = t // NQ
    npair = od // P
    return max(2 * npair * (nb - 1), 1)


def _ematB_np(t=T, od=OD):
    import ml_dtypes
    e = np.zeros((33, P), ml_dtypes.bfloat16)
    e[0, 0:DK] = 1.0
    e[32, DK:P] = 1.0
    return e


def make_in_maps(x, Wq, Wk, Wv, Wo):
    import ml_dtypes

    bf = ml_dtypes.bfloat16
    x = np.asarray(x, np.float32)
    msk = _masks_np()
    emat = _emat_np(_emat_rows())
    in_maps = []
    for c in range(N_CORES):
        b, g = c // (N_CORES // 2), c % (N_CORES // 2)
        hs = slice(OD * g, OD * (g + 1))
        in_maps.append({
            "xT": np.ascontiguousarray(x[b].T).astype(bf),
            "wqT": np.ascontiguousarray(np.asarray(Wq, np.float32)[hs, :].T).astype(bf),
            "wkT": np.ascontiguousarray(np.asarray(Wk, np.float32)[hs, :].T).astype(bf),
            "wvT": np.ascontiguousarray(np.asarray(Wv, np.float32)[hs, :].T).astype(bf),
            "woT": np.ascontiguousarray(np.asarray(Wo, np.float32)[:, hs].T).astype(bf),
            "masks": msk,
            "emat": emat,
            "ematB": _ematB_np(),
        })
    return in_maps


def combine_outputs(results):
    ng = N_CORES // 2
    out = np.empty((2, T, D), np.float32)
    for b in range(2):
        acc = results[b * ng]["y"].astype(np.float32)
        for g in range(1, ng):
            acc = acc + results[b * ng + g]["y"].astype(np.float32)
        out[b] = acc
    return out


def kernel(x, Wq, Wk, Wv, Wo):
    from concourse.bass_utils import run_bass_kernel_spmd

    nc = _get_nc()
    in_maps = make_in_maps(x, Wq, Wk, Wv, Wo)
    res = run_bass_kernel_spmd(nc, in_maps, list(range(N_CORES)))
    return combine_outputs(res.results)



# revision 45
# speedup vs baseline: 1.0284x; 1.0032x over previous
"""Multi-head causal self-attention on 8 Trainium2 NeuronCores.

Sharding: tensor-parallel over heads (4 heads/core) x data-parallel over
batch (B=2): core c -> batch c//4, head-group c%4. Each core computes its
4 heads' attention plus a partial output projection; the host sums the 4
partials per batch element.

Layout strategy (per core):
  - x is fed pre-transposed (xT: [D, T]) so QKV projections produce
    qT/kT ([head_dim, T], head-dim on partitions) and v ([T, head_dim])
    directly, with no on-device transposes anywhere.
  - Startup: the critical-path DMA (wk/wq/xc-block0 in k-chunk order,
    rotated across three issue queues) goes out first so the QKV matmuls
    start as soon as chunk 0 lands and stream behind the DMA; wv / the
    other x blocks / wo / normalization constants follow in need-order.
  - Scores are computed transposed (k on partitions, q on free dim):
    psum[k, q] = kT_tile.T @ qT_block. Two heads run concurrently via
    row-tiled tile_position (dk=64 each) into one 2-bank psum tile, so
    one Exp activation covers the pair (halves ACT call overhead).
  - Softmax skips max-subtraction (scores are bounded well inside fp32
    exp range); exp runs on ScalarE with scale=1/sqrt(dk) folded in.
    Causal masking multiplies only diagonal tiles by a 0/1 mask, one
    head on VectorE and one on GpSimd.
  - P@V uses an M=65 stationary [v_head | ones] so the softmax
    denominators accumulate in psum row 64 of the same matmul.
  - Normalization: denominator rows are bounced via tiny DMAs into
    collector tiles (always read at partition base 0 — the custom-DVE
    fast reciprocal mishandles shifted bases on hardware), inverted
    with the fast Newton-Raphson reciprocal, broadcast across a full
    128-partition head-pair by a one-hot-selector matmul, and
    multiplied in with one DVE op per pair.
  - The PE clock gate (HAM) needs ~3.4us of sustained busy to reach
    2.4GHz; dummy matmuls on a zeroed scratch tile pre-warm it during
    the DMA-bound startup and bridge feed holes in the warm-up.
  - The exp chain paces the attention phase, so the PE stream is kept
    stall-free to hold the HAM clock at 2.4GHz: the m-loop is software
    pipelined (scores of step m+1 issue before P@V of step m), the two
    head pairs' blocks alternate, and QKV / output-projection matmul
    groups are interleaved into the attention phase as PE filler.
  - Matmul operands are bf16 (1 cycle/column on the PE; fp32r costs 2).
    Accumulation is fp32 in PSUM; denominators/reciprocals stay fp32.
  - y partials leave as bf16 (halves output DMA); the host sums in fp32.
"""

import sys

for _p in ("/opt/trn_rl_repo",):
    if _p not in sys.path:
        sys.path.append(_p)

import numpy as np

P = 128
T = 2048
D = 1024
OD = 256  # output dims per core = 4 heads x 64
DK = 64
NQ = 512  # q-block (psum free size)
N_CORES = 8

_CACHE = {}


def _build_nc(t=T, d=D, od=OD):
    import concourse.bass as bass
    import concourse.tile as tile
    from concourse import bacc, mybir

    f32 = mybir.dt.float32
    f32r = mybir.dt.float32r
    bf16 = mybir.dt.bfloat16

    kt = d // P        # k-tiles over d_model
    tt = t // P        # token tiles
    nb = t // NQ       # q blocks
    npair = od // P    # head pairs (2 heads per 128 partitions)
    dpb = NQ // P      # diagonal k-tiles per q block
    nh = od // DK      # heads per core

    nslotsA = 2 * npair * (nb - 1)  # (pair, j, head) slots with j < nb-1
    npairA = nslotsA // 2           # (pair, j) pair-slots with j < nb-1
    nslotsB = 2 * npair             # slots with j == nb-1
    nrows = max(nslotsA, 1)
    nrowsB = 32 * (nslotsB - 1) + 1  # batch-B rows live at partitions 32*i

    nc = bacc.Bacc("TRN2", target_bir_lowering=False, debug=False)

    xT = nc.dram_tensor("xT", [d, t], bf16, kind="ExternalInput")
    wqT = nc.dram_tensor("wqT", [d, od], bf16, kind="ExternalInput")
    wkT = nc.dram_tensor("wkT", [d, od], bf16, kind="ExternalInput")
    wvT = nc.dram_tensor("wvT", [d, od], bf16, kind="ExternalInput")
    woT = nc.dram_tensor("woT", [od, d], bf16, kind="ExternalInput")
    masks = nc.dram_tensor("masks", [P, 2 * P], bf16, kind="ExternalInput")
    emat = nc.dram_tensor("emat", [nrows, npairA * P], bf16, kind="ExternalInput")
    ematB = nc.dram_tensor("ematB", [33, P], bf16, kind="ExternalInput")
    y = nc.dram_tensor("y", [t, d], bf16, kind="ExternalOutput")

    Exp = mybir.ActivationFunctionType.Exp
    scale = 1.0 / float(np.sqrt(DK))

    with tile.TileContext(nc) as tc:
        with (
            tc.tile_pool(name="const", bufs=1) as cpool,
            tc.tile_pool(name="qk", bufs=2 * npair * nb) as qkpool,
            tc.tile_pool(name="vp", bufs=tt) as vpool,
            tc.tile_pool(name="ht", bufs=npair * nb) as hpool,
            tc.tile_pool(name="hu", bufs=npair * nb) as hupool,
            tc.tile_pool(name="work", bufs=8) as wpool,
            tc.tile_pool(name="psS", bufs=2, space="PSUM") as psS,
            tc.tile_pool(name="psH", bufs=2, space="PSUM") as psH,
            tc.tile_pool(name="psF", bufs=2, space="PSUM") as psF,
        ):
            # ---- HAM pre-warm: the PE clock gate needs ~3.4us of sustained
            # busy to unthrottle from 1.2 to 2.4 GHz, and the DMA-fed warmup
            # can't provide it. Stream dummy matmuls over a zeroed scratch
            # tile (no DMA dependency) so the clock is hot when real work
            # lands. ----
            warm_sb = cpool.tile([P, NQ], bf16, tag="warm")
            nc.vector.memset(warm_sb[:], 0.0)
            _wi = [0]

            def warm_mms(n):
                # dummy matmuls ride the psH ring (psF belongs to the QKV
                # stream); they bridge DMA-feed holes so the HAM clock gate
                # stays at 2.4 GHz through the warm-up
                ps = psH.tile([P, NQ], f32, tag="h", name=f"warm_{_wi[0]}")
                _wi[0] += 1
                for _ in range(n):
                    nc.tensor.matmul(ps[:], warm_sb[:, 0:P], warm_sb[:], start=True, stop=True)

            warm_mms(8)

            # ---- constant tiles (DMAs for the late-needed ones issue last) ----
            wo_sb = cpool.tile([P, npair * d], bf16, tag="wo")
            mask_sb = cpool.tile([P, 2 * P], bf16, tag="mask")
            emat_sb = cpool.tile([nrows, npairA * P], bf16, tag="emat")
            ematB_sb = cpool.tile([33, P], bf16, tag="ematB")

            # x and the QKV weights live in a scoped pool released after the
            # projections, freeing space for the attention phase.
            xpool = tc.alloc_tile_pool(name="xp", bufs=1)
            xc = [xpool.tile([P, kt * NQ], bf16, tag=f"xc{c}", name=f"xc_{c}") for c in range(nb)]
            wq_sb = xpool.tile([P, kt * od], bf16, tag="wq")
            wk_sb = xpool.tile([P, kt * od], bf16, tag="wk")
            wv_sb = xpool.tile([P, kt * od], bf16, tag="wv")

            _eng = [nc.sync, nc.gpsimd, nc.scalar]
            _ei = [0]

            def _issue(dst, src):
                _eng[_ei[0] % 3].dma_start(dst, src)
                _ei[0] += 1

            xTv = xT.rearrange("(k p) q -> p k q", p=P)
            wqv = wqT.rearrange("(k p) o -> p k o", p=P)
            wkv = wkT.rearrange("(k p) o -> p k o", p=P)
            wvv = wvT.rearrange("(k p) o -> p k o", p=P)
            xdst = [xc[c][:].rearrange("p (k q) -> p k q", q=NQ) for c in range(nb)]
            wqd = wq_sb[:].rearrange("p (k o) -> p k o", o=od)
            wkd = wk_sb[:].rearrange("p (k o) -> p k o", o=od)
            wvd = wv_sb[:].rearrange("p (k o) -> p k o", o=od)

            # Startup-critical stream rides the two HWDGE queues (sync, scalar
            # = SP/Activation hardware descriptor generators): finest chunks
            # first so the first matmuls fire as soon as chunk 0 lands, then
            # coarser batches to amortize the ~0.65us per-DMA issue cost.
            # gpsimd DMAs are SWDGE (descriptor generation occupies the Pool
            # engine ~1us per transfer), so it only carries tensors needed
            # after the warm-up: wv (needed ~5us in), the late x blocks, and
            # the output-side constants.
            # The Pool (gpsimd) SWDGE path stalls its queue for >10us once it
            # has a few transfers queued, so it only carries tensors needed
            # after ~40us (x2, x3, wo, norm constants). Everything the warmup
            # touches rides the two fast HWDGE queues.
            # Each queue admits only ~4 in-flight DMAs before issues stall on
            # transfer completions, so the startup-critical bytes must sit
            # inside that window: one finest-grain chunk to unblock the first
            # matmul, then one big transfer covering the rest of the tensor.
            # The warm-up is aggregate-bandwidth-bound (~2.5MB of critical
            # bytes), so the layout only needs consumption order: wk/x0
            # finest-first on the HWDGE queues, wq on the SWDGE queue, wv
            # before x1/x2, the rest late.
            assert nb == 4 and kt == 8
            nc.sync.dma_start(xdst[0][:, 0:1], xTv[:, 0:1, 0:NQ])
            nc.scalar.dma_start(wkd[:, 0:1], wkv[:, 0:1])
            nc.gpsimd.dma_start(wqd[:, 0:1], wqv[:, 0:1])
            nc.sync.dma_start(xdst[0][:, 1:2], xTv[:, 1:2, 0:NQ])
            nc.scalar.dma_start(wkd[:, 1:3], wkv[:, 1:3])
            nc.gpsimd.dma_start(wqd[:, 1:3], wqv[:, 1:3])
            nc.sync.dma_start(xdst[0][:, 2:4], xTv[:, 2:4, 0:NQ])
            nc.scalar.dma_start(wkd[:, 3:8], wkv[:, 3:8])
            nc.gpsimd.dma_start(wqd[:, 3:8], wqv[:, 3:8])
            nc.sync.dma_start(xdst[0][:, 4:6], xTv[:, 4:6, 0:NQ])
            nc.scalar.dma_start(wvd[:, 0:4], wvv[:, 0:4])
            nc.sync.dma_start(xdst[0][:, 6:8], xTv[:, 6:8, 0:NQ])
            nc.scalar.dma_start(wvd[:, 4:8], wvv[:, 4:8])
            nc.sync.dma_start(mask_sb[:], masks[:])
            nc.sync.dma_start(xdst[1][:, 0:4], xTv[:, 0:4, NQ:2 * NQ])
            nc.scalar.dma_start(xdst[1][:, 4:8], xTv[:, 4:8, NQ:2 * NQ])
            nc.sync.dma_start(xdst[2][:, 0:4], xTv[:, 0:4, 2 * NQ:3 * NQ])
            nc.scalar.dma_start(xdst[2][:, 4:8], xTv[:, 4:8, 2 * NQ:3 * NQ])
            nc.gpsimd.dma_start(xdst[3][:, 0:4], xTv[:, 0:4, 3 * NQ:4 * NQ])
            nc.gpsimd.dma_start(xdst[3][:, 4:8], xTv[:, 4:8, 3 * NQ:4 * NQ])
            for pp in range(npair):
                nc.gpsimd.dma_start(wo_sb[:, pp * d:(pp + 1) * d], woT[pp * P:(pp + 1) * P, :])
            nc.gpsimd.dma_start(emat_sb[:], emat[:])
            nc.gpsimd.dma_start(ematB_sb[:], ematB[:])

            # ---- persistent tiles ----
            qT = [[qkpool.tile([P, NQ], bf16, tag="qT", name=f"qT_{pp}_{n}") for n in range(nb)] for pp in range(npair)]
            kT = [[qkpool.tile([P, NQ], bf16, tag="kT", name=f"kT_{pp}_{n}") for n in range(nb)] for pp in range(npair)]
            v_sb = [vpool.tile([P, nh * (DK + 1)], bf16, tag="v", name=f"v_{tk}") for tk in range(tt)]
            hT = [[hpool.tile([P, NQ], bf16, tag="hT", name=f"hT_{pp}_{n}") for n in range(nb)] for pp in range(npair)]
            hu = {}

            sumsA = wpool.tile([max(nslotsA, 1), NQ], f32, tag="sumsA", bufs=1)
            # one [33, NQ] collector per pair so the custom-DVE
            # reciprocal_approx_fast always reads at partition offset 0
            # (its microcode mishandles shifted input bases on hardware)
            sumsB = [wpool.tile([33, NQ], f32, tag=f"sumsB{pp}", bufs=1, name=f"sumsB_{pp}") for pp in range(npair)]
            for pp in range(npair):
                nc.vector.memset(sumsB[pp][:], 1.0)
            batchA = []  # (pp, j, h) in collector-row order
            batchB = []

            # ---- emit helpers ----
            def emit_kq(pp, n, which=(0, 1)):
                for idx, (dst, w_sb) in enumerate(((kT, wk_sb), (qT, wq_sb))):
                    if idx not in which:
                        continue
                    ps = psF.tile([P, NQ], f32, tag="acc", name=f"kqps_{pp}_{n}_{idx}")
                    for k in range(kt):
                        nc.tensor.matmul(
                            ps[:],
                            w_sb[:, k * od + pp * P: k * od + (pp + 1) * P],
                            xc[n][:, k * NQ:(k + 1) * NQ],
                            start=(k == 0),
                            stop=(k == kt - 1),
                        )
                    nc.vector.tensor_copy(dst[pp][n][:], ps[:])

            def emit_v(tk):
                # each head's 64 v-columns are followed by a ones column so
                # the P@V matmul also accumulates the softmax denominator
                vv = v_sb[tk][:].rearrange("p (h c) -> p h c", c=DK + 1)
                nc.vector.memset(v_sb[tk][:], 1.0)
                ps = psF.tile([P, od], f32, tag="acc", name=f"vps_{tk}")
                for k in range(kt):
                    nc.tensor.matmul(
                        ps[:],
                        xc[tk // dpb][:, k * NQ + (tk % dpb) * P: k * NQ + (tk % dpb + 1) * P],
                        wv_sb[:, k * od:(k + 1) * od],
                        start=(k == 0),
                        stop=(k == kt - 1),
                    )
                nc.vector.tensor_copy(
                    vv[:, :, 0:DK],
                    ps[:].rearrange("p (h c) -> p h c", c=DK),
                )

            # ---- granule variants: one matmul (or one small op) per call so
            # the filler drip can fill sub-microsecond PE gaps between the
            # scores issue and the exp-gated P@V ----
            def kq_granules(pp, n, idx):
                dst, w_sb = ((kT, wk_sb), (qT, wq_sb))[idx]
                st = {}
                gs = []
                for k in range(kt):
                    def gk(k=k):
                        if k == 0:
                            st["ps"] = psF.tile([P, NQ], f32, tag="acc", name=f"kqps_{pp}_{n}_{idx}")
                        nc.tensor.matmul(
                            st["ps"][:],
                            w_sb[:, k * od + pp * P: k * od + (pp + 1) * P],
                            xc[n][:, k * NQ:(k + 1) * NQ],
                            start=(k == 0),
                            stop=(k == kt - 1),
                        )
                    gs.append(gk)
                gs.append(lambda: nc.vector.tensor_copy(dst[pp][n][:], st["ps"][:]))
                return gs

            def v_granules(tk):
                st = {}
                gs = []
                for k in range(kt):
                    def gk(k=k):
                        if k == 0:
                            nc.gpsimd.memset(v_sb[tk][:], 1.0)
                            st["ps"] = psF.tile([P, od], f32, tag="acc", name=f"vps_{tk}")
                        nc.tensor.matmul(
                            st["ps"][:],
                            xc[tk // dpb][:, k * NQ + (tk % dpb) * P: k * NQ + (tk % dpb + 1) * P],
                            wv_sb[:, k * od:(k + 1) * od],
                            start=(k == 0),
                            stop=(k == kt - 1),
                        )
                    gs.append(gk)

                def gc():
                    vv = v_sb[tk][:].rearrange("p (h c) -> p h c", c=DK + 1)
                    nc.vector.tensor_copy(vv[:, :, 0:DK], st["ps"][:].rearrange("p (h c) -> p h c", c=DK))
                gs.append(gc)
                return gs

            invA_holder = {}

            def emit_normA_recip(nslots):
                invf = wpool.tile([nslots, NQ], f32, tag="invAf", bufs=1, name="inv_Af")
                nc.vector.reciprocal_approx_fast(invf[:], sumsA[0:nslots, :])
                inv = wpool.tile([nslots, NQ], bf16, tag="invA", bufs=1, name="inv_A")
                nc.vector.tensor_copy(inv[:], invf[:])
                invA_holder["inv"] = inv

            def normA_pair_granules(ti, nslots):
                # pair-slot ti covers batchA rows 2*ti (head 0) and 2*ti+1
                # (head 1); one broadcast matmul + one mul handle all 128
                # partitions of the (pp, j) block
                pp, j, _ = batchA[2 * ti]
                st = {}

                def g0():
                    inv = invA_holder["inv"]
                    st["psb"] = psF.tile([P, NQ], f32, tag="acc", name=f"psb_A_{ti}")
                    nc.tensor.matmul(st["psb"][:], emat_sb[0:nslots, ti * P:(ti + 1) * P], inv[:], start=True, stop=True)

                def g1():
                    nc.vector.tensor_mul(hT[pp][j][:], hu[(pp, j)][:], st["psb"][:])
                return [g0, g1]

            def normB_granules(pp, ps_pool=None):
                # pair pp's two denominators live at rows 0 and 32 of its
                # own collector; ematB broadcasts row 0 -> partitions 0:64
                # and row 32 -> partitions 64:128
                st = {}

                def gr():
                    invf = wpool.tile([33, NQ], f32, tag="invBf", bufs=2, name=f"invBf_{pp}")
                    nc.vector.reciprocal_approx_fast(invf[:], sumsB[pp][0:33, :])
                    st["invB"] = wpool.tile([33, NQ], bf16, tag="invB", bufs=2, name=f"invB_{pp}")
                    nc.vector.tensor_copy(st["invB"][:], invf[:])

                def gm():
                    st["psb"] = (ps_pool or psF).tile([P, NQ], f32, tag="acc", name=f"psbB_{pp}")
                    nc.tensor.matmul(st["psb"][:], ematB_sb[0:33, :], st["invB"][:], start=True, stop=True)

                def gu():
                    nc.vector.tensor_mul(hT[pp][nb - 1][:], hu[(pp, nb - 1)][:], st["psb"][:])
                return [gr, gm, gu]

            obw = min(NQ, d)

            def oproj_granules(tk, use_act=False):
                gs = []
                for ob in range(d // obw):
                    st = {}

                    def gm(ob=ob, st=st):
                        st["psy"] = psF.tile([P, obw], f32, tag="acc", name=f"yps_{tk}_{ob}")
                        for pp in range(npair):
                            nc.tensor.matmul(
                                st["psy"][:],
                                hT[pp][tk // dpb][:, (tk % dpb) * P:(tk % dpb + 1) * P],
                                wo_sb[:, pp * d + ob * obw: pp * d + (ob + 1) * obw],
                                start=(pp == 0),
                                stop=(pp == npair - 1),
                            )

                    def gc(ob=ob, st=st):
                        ysb = wpool.tile([P, obw], bf16, tag="ysb", bufs=4)
                        if use_act:
                            # tail only: mid-phase the scalar queue is the
                            # exp pacer and must not carry copies
                            nc.scalar.copy(ysb[:], st["psy"][:])
                        else:
                            nc.vector.tensor_copy(ysb[:], st["psy"][:])
                        nc.sync.dma_start(y[tk * P:(tk + 1) * P, ob * obw:(ob + 1) * obw], ysb[:])
                    gs += [gm, gc]
                return gs

            # ---- warm-up: first-block QKV emitted eagerly; the scheduler
            # starts each piece as its DMA chunks land. Dummy matmuls woven
            # between the groups keep the PE busy across feed holes. ----
            emit_kq(0, 0)
            for tk in range(min(dpb, tt)):
                emit_v(tk)
            for pp in range(1, npair):
                emit_kq(pp, 0)

            # ---- attention m-step: both heads' scores land in one 2-bank
            # psum tile so a single Exp covers the pair; only the 128-wide
            # diagonal boundary needs the causal mask ----
            def emit_step(pp, j, m):
                dlt = m - dpb * j
                lo = max(dlt, 0) * P  # first live q column of this k-tile
                pss = psS.tile([P, 2 * NQ], f32, tag="acc", name=f"pss_{pp}_{j}_{m}")
                ps3 = pss[:].rearrange("p (h q) -> p h q", q=NQ)
                for h in range(2):
                    nc.tensor.matmul(
                        ps3[:, h, lo:],
                        kT[pp][m // dpb][h * DK:(h + 1) * DK, (m % dpb) * P:(m % dpb + 1) * P],
                        qT[pp][j][h * DK:(h + 1) * DK, lo:],
                        start=True,
                        stop=True,
                        tile_position=(h * DK, 0),
                    )
                e = wpool.tile([P, 2 * NQ], bf16, tag="exp", bufs=12)
                e3 = e[:].rearrange("p (h q) -> p h q", q=NQ)
                nc.scalar.activation(e3[:, :, lo:], ps3[:, :, lo:], Exp, bias=0.0, scale=scale)
                if dlt >= 0:
                    # one DVE op masks both heads (mask stored doubled)
                    nc.vector.tensor_mul(
                        e3[:, :, lo:lo + P],
                        e3[:, :, lo:lo + P],
                        mask_sb[:].rearrange("p (h q) -> p h q", q=P),
                    )
                return (m, lo, e3)

            def _emit_pv(pp, psh, nm, m, lo, e3):
                for h in range(2):
                    hh = 2 * pp + h
                    nc.tensor.matmul(
                        psh[h][0:DK + 1, lo:],
                        v_sb[m][:, hh * (DK + 1): (hh + 1) * (DK + 1)],
                        e3[:, h, lo:],
                        start=(m == 0),
                        stop=(m == nm - 1),
                    )

            def finish_block(pp, j, psh):
                # denominator rows first (they gate the batched reciprocal),
                # then the wider hu copies
                for h in range(2):
                    key = (pp, j, h)
                    if j < nb - 1:
                        row = len(batchA)
                        batchA.append(key)
                        stmp = wpool.tile([1, NQ], f32, tag="stmp", bufs=3)
                        nc.vector.tensor_copy(stmp[:], psh[h][DK:DK + 1, :])
                        # single-descriptor bounce: cheap on the SWDGE queue,
                        # and it keeps the sync queue free for y output tiles
                        nc.gpsimd.dma_start(sumsA[row:row + 1, :], stmp[:])
                    else:
                        batchB.append(key)
                        nc.vector.tensor_copy(sumsB[pp][32 * h:32 * h + 1, :], psh[h][DK:DK + 1, :])
                key = (pp, j)
                hu[key] = hupool.tile([P, NQ], bf16, tag="hu", name=f"hu_{pp}_{j}")
                for h in range(2):
                    if j == nb - 1 and pp == npair - 1 and h == 0:
                        # tail, unshifted half: ACT is idle there. (The h=1
                        # half needs a partition-shifted write, which only
                        # DVE handles on hardware.)
                        nc.scalar.copy(hu[key][0:DK, :], psh[h][0:DK, :])
                    else:
                        nc.vector.tensor_copy(hu[key][h * DK:(h + 1) * DK, :], psh[h][0:DK, :])

            released_x = False
            for j in range(nb):
                filler = []
                if j > 0:
                    # this block's own later v tiles (needed from m = dpb*j)
                    for tk in range(dpb * j, min(dpb * (j + 1), tt)):
                        filler.append(lambda tk=tk: emit_v(tk))
                if j < nb - 1:
                    for pp in range(npair):
                        filler.append(lambda pp=pp, n=j + 1: emit_kq(pp, n, (0,)))
                        filler.append(lambda pp=pp, n=j + 1: emit_kq(pp, n, (1,)))
                if j == nb - 1:
                    if nslotsA:
                        filler.append(lambda: emit_normA_recip(nslotsA))
                        # per q-block: its 2 normalization pair-slots, then
                        # the output-projection token blocks they unlock; the
                        # last two units stay reserved as PE work covering
                        # the tail's reciprocal window
                        for b in range(nb - 1):
                            for ti in (2 * b, 2 * b + 1):
                                filler += normA_pair_granules(ti, nslotsA)
                            for tk in range(dpb * b, dpb * (b + 1)):
                                if tk < dpb * (nb - 1) - 2:
                                    filler += oproj_granules(tk)
                nm = dpb * (j + 1)
                nsteps = npair * nm
                fstate = [0, 0, nsteps]  # steps done, fillers emitted, total
                for pp in range(npair):
                    psh = [psH.tile([P, NQ], f32, tag="h", name=f"psh_{pp}_{j}_{h}") for h in range(2)]
                    pending = None
                    for m in range(nm):
                        step = emit_step(pp, j, m)
                        # drip fillers between the scores issue and the
                        # exp-gated P@V so the PE crosses the exp latency
                        # without an exposed pipeline-drain boundary
                        fstate[0] += 1
                        while fstate[1] < len(filler) and fstate[1] < fstate[0] * len(filler) // max(fstate[2], 1):
                            filler[fstate[1]]()
                            fstate[1] += 1
                        if pending is not None:
                            _emit_pv(pp, psh, nm, *pending)
                        pending = step
                    _emit_pv(pp, psh, nm, *pending)
                    finish_block(pp, j, psh)
                    if j == nb - 1 and pp < npair - 1:
                        # queue this pair's normalization as filler so it
                        # drips into the next pair's steps
                        filler += normB_granules(pp)
                while fstate[1] < len(filler):
                    filler[fstate[1]]()
                    fstate[1] += 1
                if j >= nb - 2 and not released_x:
                    released_x = True
                    xpool.release()

            # ---- tail: last pair's normalization + last block's output
            # projection, software-pipelined 4 psum buffers deep. The first
            # pair-0 half-accumulations issue before the normalization chain
            # so the PE stays busy (and HAM-warm) while it resolves.
            units = [(tk, ob) for tk in range(dpb * (nb - 1), tt) for ob in range(d // obw)]
            psys = {}

            def tail_p0(u):
                tk, ob = units[u]
                pl, tg, w = ((psF, "acc", obw), (psH, "h", NQ))[u % 2]
                psy = pl.tile([P, w], f32, tag=tg, name=f"yt_{u}")
                nc.tensor.matmul(
                    psy[:, 0:obw],
                    hT[0][nb - 1][:, (tk % dpb) * P:(tk % dpb + 1) * P],
                    wo_sb[:, ob * obw:(ob + 1) * obw],
                    start=True, stop=False,
                )
                psys[u] = psy

            def tail_p1(u):
                tk, ob = units[u]
                psy = psys.pop(u)
                nc.tensor.matmul(
                    psy[:, 0:obw],
                    hT[1][nb - 1][:, (tk % dpb) * P:(tk % dpb + 1) * P],
                    wo_sb[:, d + ob * obw: d + (ob + 1) * obw],
                    start=False, stop=True,
                )
                ysb = wpool.tile([P, obw], bf16, tag="ysb", bufs=4)
                if u % 2:
                    nc.scalar.copy(ysb[:], psy[:, 0:obw])
                    # split the tail's output stream over both HWDGE queues
                    nc.scalar.dma_start(y[tk * P:(tk + 1) * P, ob * obw:(ob + 1) * obw], ysb[:])
                else:
                    nc.vector.tensor_copy(ysb[:], psy[:, 0:obw])
                    nc.sync.dma_start(y[tk * P:(tk + 1) * P, ob * obw:(ob + 1) * obw], ysb[:])

            # reserved block-(nb-2) oproj units first (they own the psF ring
            # before tail_p0 claims it), then the pair-0 halves — together
            # they keep the PE streaming through the reciprocal. ACT is idle
            # from here on, so their psum->sbuf copies ride the scalar queue.
            for tk in range(dpb * (nb - 1) - 2, dpb * (nb - 1)):
                for g in oproj_granules(tk, use_act=True):
                    g()
            for u in range(4):
                tail_p0(u)
            for g in normB_granules(npair - 1, ps_pool=psS):
                g()
            for u in range(4):
                tail_p1(u)
            # bridge the p1->p0 ring turnaround (a recurring ~0.9us PE gap
            # that re-throttles the clock gate right before the final
            # output-projection burst). Placed after the normalization chain
            # so it never delays the psb matmul that gates the p1 units.
            ps_wt = psS.tile([P, NQ], f32, tag="acc", name="warm_tail")
            for _ in range(4):
                nc.tensor.matmul(ps_wt[:], warm_sb[:, 0:P], warm_sb[:], start=True, stop=True)
            for u in range(4, len(units)):
                tail_p0(u)
            for u in range(4, len(units)):
                tail_p1(u)

    nc.compile()
    return nc


def _get_nc():
    if "nc" not in _CACHE:
        _CACHE["nc"] = _build_nc()
    return _CACHE["nc"]


def _emat_np(nrows):
    # pair-slot ti broadcasts row 2*ti across partitions 0:64 and row
    # 2*ti+1 across partitions 64:128 of its 128-wide column block
    import ml_dtypes
    e = np.zeros((nrows, (nrows // 2) * P), ml_dtypes.bfloat16)
    for ti in range(nrows // 2):
        e[2 * ti, ti * P: ti * P + DK] = 1.0
        e[2 * ti + 1, ti * P + DK: (ti + 1) * P] = 1.0
    return e


def _masks_np():
    import ml_dtypes
    kk = np.arange(P)[:, None]
    qq = np.arange(P)[None, :]
    m = (kk <= qq).astype(ml_dtypes.bfloat16)
    return np.concatenate([m, m], axis=1)


def _emat_rows(t=T, od=OD):
    nb = t // NQ
    npair = od // P
    return max(2 * npair * (nb - 1), 1)


def _ematB_np(t=T, od=OD):
    import ml_dtypes
    e = np.zeros((33, P), ml_dtypes.bfloat16)
    e[0, 0:DK] = 1.0
    e[32, DK:P] = 1.0
    return e


def make_in_maps(x, Wq, Wk, Wv, Wo):
    import ml_dtypes

    bf = ml_dtypes.bfloat16
    x = np.asarray(x, np.float32)
    msk = _masks_np()
    emat = _emat_np(_emat_rows())
    in_maps = []
    for c in range(N_CORES):
        b, g = c // (N_CORES // 2), c % (N_CORES // 2)
        hs = slice(OD * g, OD * (g + 1))
        in_maps.append({
            "xT": np.ascontiguousarray(x[b].T).astype(bf),
            "wqT": np.ascontiguousarray(np.asarray(Wq, np.float32)[hs, :].T).astype(bf),
            "wkT": np.ascontiguousarray(np.asarray(Wk, np.float32)[hs, :].T).astype(bf),
            "wvT": np.ascontiguousarray(np.asarray(Wv, np.float32)[hs, :].T).astype(bf),
            "woT": np.ascontiguousarray(np.asarray(Wo, np.float32)[:, hs].T).astype(bf),
            "masks": msk,
            "emat": emat,
            "ematB": _ematB_np(),
        })
    return in_maps


def combine_outputs(results):
    ng = N_CORES // 2
    out = np.empty((2, T, D), np.float32)
    for b in range(2):
        acc = results[b * ng]["y"].astype(np.float32)
        for g in range(1, ng):
            acc = acc + results[b * ng + g]["y"].astype(np.float32)
        out[b] = acc
    return out


def kernel(x, Wq, Wk, Wv, Wo):
    from concourse.bass_utils import run_bass_kernel_spmd

    nc = _get_nc()
    in_maps = make_in_maps(x, Wq, Wk, Wv, Wo)
    res = run_bass_kernel_spmd(nc, in_maps, list(range(N_CORES)))
    return combine_outputs(res.results)

